# revision 1
# baseline (speedup 1.0000x reference)
"""CfC head (mLSTM-style scan) Trainium2 kernel.

Math (per timestep t, per (b,h)):
    pre_g = xt*Wg_w + Wg_b            (xt = (x_codes-65)/100)
    i_t = exp(pre_i - n), f_t = exp(pre_f - n), o_t = exp(pre_o - n)
    g_t = sigmoid(pre_g); lam = sigmoid(pre_l)
    c   = f_t*c + i_t*g_t
    h   = (h + DT*o_t*sigmoid(c)) / (1 + DT*lam)
    n  += 0.01*(i_t + f_t + o_t - 3)
    y_t = h @ proj_w.T + proj_b

Device mapping: H=1024 sharded over 8 cores (128 h-values per core, one SBUF
partition each); free dim packs (batch-major, time-minor) blocks of TB steps.
The n-recurrence is handled per block by tracking the within-block drift
delta = n - n_blockstart, linearized as the affine scan
    delta_t = (1 - 0.01*P_t) * delta_{t-1} + (0.01*P_t - 0.03),
    P_t = (Ei+Ef+Eo)_t * exp(-n_blockstart),
which runs as one tensor_tensor_scan over the whole block (validated: rel err
1.4e-4 at TB=32 vs exact). c and h are exact affine scans given en = exp(-n):
    c_t = (Ef_t*en) * c_{t-1} + (Ei_t*G_t*en)
    h_t = L1_t * h_{t-1} + L1_t*DT*Eo_t*en*sigmoid(c_t),  L1 = 1/(1+DT*lam)
L1 uses the Neumann form 1 - q + q^2 = (q-0.5)^2 + 0.75 (q = DT*lam <= 0.01).
Sigmoids use tanh so every activation (exp/tanh/square) lives in the single
"exp_and_others" ACT table set (no table reloads).

Most tiles are fp16 (DVE 2x tensor_tensor / 4x tensor_scalar modes); Sq, L1
and h stay fp32 — L1 is the h-scan decay rate whose error is amplified by
1/(1-L1) ~ 200x, and h feeds the output directly (validated numerically:
rel err 4.3e-4 vs reference with this dtype split).

y partials (over each core's 128 h) are accumulated on PE into PSUM and
summed across cores on the host.
"""

import os
from contextlib import ExitStack

import numpy as np

import concourse.bacc as bacc
import concourse.mybir as mybir
import concourse.tile as tile
from concourse.bass_utils import run_bass_kernel_spmd

AF = mybir.ActivationFunctionType
OP = mybir.AluOpType
F32 = mybir.dt.float32
F16 = mybir.dt.float16

B, S, H = 64, 2048, 1024
NCORES = 8
HC = H // NCORES  # 128 h-values per core = partition dim
DT = 0.01

TB = int(os.environ.get("KERNEL_TB", "32"))  # timesteps per block
CCLAMP = 3.0e4  # c-carry clamp; sigmoid(c>=17) == 1.0f so this is exact

_cached = {}
_last_results = None


def build_program(s=S, tb=TB):
    nb = s // tb
    nfd = B * tb           # free dim of block tiles, (b-major, t-minor)
    nslab = nfd // 128     # 128-wide matmul slabs per block

    nc = bacc.Bacc(
        "TRN2", target_bir_lowering=False, debug=False, num_devices=NCORES
    )
    x_d = nc.dram_tensor("x", [B, s], F16, kind="ExternalInput").ap()
    wv_d = nc.dram_tensor("wv", [HC, 10], F32, kind="ExternalInput").ap()
    pj_d = nc.dram_tensor("projT", [HC, 2], F32, kind="ExternalInput").ap()
    n0_d = nc.dram_tensor("n0", [HC, 1], F32, kind="ExternalInput").ap()
    y_d = nc.dram_tensor("yout", [nb, 128, tb], F32, kind="ExternalOutput").ap()

    def r3(ap):  # [128, nfd] -> [128, B, tb]
        return ap.rearrange("p (b t) -> p b t", t=tb)

    with tile.TileContext(nc) as tc, ExitStack() as ctx:
        wp = ctx.enter_context(tc.tile_pool(name="w", bufs=1))
        pha = ctx.enter_context(tc.tile_pool(name="pha", bufs=2))
        chn = ctx.enter_context(tc.tile_pool(name="chn", bufs=1))
        sm = ctx.enter_context(tc.tile_pool(name="sm", bufs=2))
        pp = ctx.enter_context(tc.tile_pool(name="pp", bufs=2, space="PSUM"))

        wv = wp.tile([HC, 10], F32)
        nc.sync.dma_start(wv[:], wv_d)
        pj = wp.tile([HC, 2], F32)
        nc.sync.dma_start(pj[:], pj_d)
        n0t = wp.tile([HC, 1], F32)
        nc.sync.dma_start(n0t[:], n0_d)

        # carries: n at block start (per h,b), exp(-n), c, h
        Nc = wp.tile([HC, B], F32)
        nc.vector.memset(Nc[:], 0.0)
        nc.vector.tensor_scalar(Nc[:], Nc[:], n0t[:, 0:1], None, OP.add)
        ENc = wp.tile([HC, B], F16)
        nc.scalar.activation(ENc[:], Nc[:], AF.Exp, scale=-1.0)
        cz = wp.tile([HC, B], F16)
        nc.vector.memset(cz[:], 0.0)
        hz = wp.tile([HC, B], F32)
        nc.vector.memset(hz[:], 0.0)
        Cc_v, Hc_v = cz[:], hz[:]
        bqm = wp.tile([HC, 1], F32)
        nc.vector.memset(bqm[:], DT / 2 - 0.5)

        for k in range(nb):
            t0 = k * tb
            X = pha.tile([128, nfd], F16, tag="X")
            nc.sync.dma_start(
                r3(X[:]), x_d[:, t0 : t0 + tb].partition_broadcast(128)
            )
            # gate pre-activations, fused through ACT scale/bias
            Ei = pha.tile([128, nfd], F16, tag="Ei")
            nc.scalar.activation(
                Ei[:], X[:], AF.Exp, bias=wv[:, 1:2], scale=wv[:, 0:1]
            )
            Ef = pha.tile([128, nfd], F16, tag="Ef")
            nc.scalar.activation(
                Ef[:], X[:], AF.Exp, bias=wv[:, 3:4], scale=wv[:, 2:3]
            )
            Eo = pha.tile([128, nfd], F16, tag="Eo")
            nc.scalar.activation(
                Eo[:], X[:], AF.Exp, bias=wv[:, 5:6], scale=wv[:, 4:5]
            )
            Tg = pha.tile([128, nfd], F16, tag="Tg")
            nc.scalar.activation(
                Tg[:], X[:], AF.Tanh, bias=wv[:, 7:8], scale=wv[:, 6:7]
            )
            Tl = pha.tile([128, nfd], F16, tag="Tl")
            nc.scalar.activation(
                Tl[:], X[:], AF.Tanh, bias=wv[:, 9:10], scale=wv[:, 8:9]
            )

            # G = 0.5*Tg+0.5 ; EiG = Ei*G  (both land in Tg)
            nc.vector.tensor_scalar(Tg[:], Tg[:], 0.5, 0.5, OP.mult, OP.add)
            nc.vector.tensor_mul(Tg[:], Ei[:], Tg[:])
            # Esum = Ei+Ef+Eo, then P = Esum*exp(-Nc)  (lands in Ei)
            nc.vector.tensor_add(Ei[:], Ei[:], Ef[:])
            nc.vector.tensor_add(Ei[:], Ei[:], Eo[:])
            ENc_bc = ENc[:].unsqueeze(2).broadcast_to([HC, B, tb])
            nc.vector.tensor_mul(r3(Ei[:]), r3(Ei[:]), ENc_bc)

            # delta scan: delta = (1-0.01P)*prev + (0.01P-0.03)
            a = chn.tile([128, nfd], F16, tag="a")
            nc.vector.tensor_scalar(a[:], Ei[:], -0.01, 1.0, OP.mult, OP.add)
            rr = chn.tile([128, nfd], F16, tag="r")
            nc.vector.tensor_scalar(rr[:], Ei[:], 0.01, -0.03, OP.mult, OP.add)
            nc.vector.memset(r3(a[:])[:, :, 0], 0.0)
            d = chn.tile([128, nfd], F16, tag="d")
            nc.vector.tensor_tensor_scan(d[:], a[:], rr[:], 0.0, OP.mult, OP.add)
            nc.vector.tensor_add(Nc[:], Nc[:], r3(d[:])[:, :, tb - 1])

            # EN = exp(-(Nc_old + delta_{t-1})): shifted exp, slots = 1, * ENc
            ED = chn.tile([128, nfd], F16, tag="ED")
            nc.scalar.activation(ED[:, 1:nfd], d[:, 0 : nfd - 1], AF.Exp, scale=-1.0)
            nc.vector.memset(r3(ED[:])[:, :, 0], 1.0)
            nc.vector.tensor_mul(r3(ED[:]), r3(ED[:]), ENc_bc)

            # c scan: a_c = Ef*EN (in Ef), b_c = EiG*EN (in Tg)
            nc.vector.tensor_mul(Ef[:], Ef[:], ED[:])
            nc.vector.tensor_mul(Tg[:], Tg[:], ED[:])
            t64 = sm.tile([HC, B], F16, tag="t64")
            nc.vector.tensor_mul(t64[:], r3(Ef[:])[:, :, 0], Cc_v)
            nc.vector.tensor_add(
                r3(Tg[:])[:, :, 0], r3(Tg[:])[:, :, 0], t64[:]
            )
            nc.vector.memset(r3(Ef[:])[:, :, 0], 0.0)
            c = chn.tile([128, nfd], F16, tag="c")
            nc.vector.tensor_tensor_scan(c[:], Ef[:], Tg[:], 0.0, OP.mult, OP.add)

            # sigmoid(c) via tanh; L1 = 1-q+q^2 = (q-0.5)^2+0.75 with
            # q = DT*lam = DT/2*(Tl+1): fold q into the Square ACT directly:
            # Sq = (DT/2*Tl + (DT/2-0.5))^2
            Tc = chn.tile([128, nfd], F16, tag="Tc")
            nc.scalar.activation(Tc[:], c[:], AF.Tanh, scale=0.5)
            Sq = chn.tile([128, nfd], F32, tag="Sq")
            nc.scalar.activation(Sq[:], Tl[:], AF.Square, bias=bqm[:], scale=DT / 2)
            L1 = chn.tile([128, nfd], F32, tag="L1")
            nc.vector.tensor_scalar(L1[:], Sq[:], 0.75, None, OP.add)
            # L1D = DT/2 * L1 on the scalar engine (fp16 out)
            L1D = chn.tile([128, nfd], F16, tag="L1D")
            nc.scalar.mul(L1D[:], L1[:], DT / 2)

            # b_h = Eo*L1D*EN*(Tc+1)   (lands in Eo)
            nc.vector.tensor_mul(Eo[:], Eo[:], L1D[:])
            nc.vector.tensor_mul(Eo[:], Eo[:], ED[:])
            nc.vector.tensor_scalar(Tc[:], Tc[:], 1.0, None, OP.add)
            nc.vector.tensor_mul(Eo[:], Eo[:], Tc[:])
            t64b = sm.tile([HC, B], F32, tag="t64b")
            nc.vector.tensor_mul(t64b[:], r3(L1[:])[:, :, 0], Hc_v)
            nc.vector.tensor_add(
                r3(Eo[:])[:, :, 0], r3(Eo[:])[:, :, 0], t64b[:]
            )
            nc.vector.memset(r3(L1[:])[:, :, 0], 0.0)
            h = chn.tile([128, nfd], F32, tag="h")
            nc.vector.tensor_tensor_scan(h[:], L1[:], Eo[:], 0.0, OP.mult, OP.add)

            # y partials: psum[m, 2j:2j+2] = h-slab_j.T @ projT
            ps = pp.tile([128, tb], F32)
            for j in range(nslab):
                nc.tensor.matmul(
                    ps[:, 2 * j : 2 * j + 2],
                    h[:, 128 * j : 128 * (j + 1)],
                    pj[:],
                    start=True,
                    stop=True,
                )
            ysb = sm.tile([128, tb], F32, tag="ysb")
            nc.scalar.copy(ysb[:], ps[:])
            nc.sync.dma_start(y_d[k], ysb[:])

            # c grows without bound for lanes with persistent f_t > 1 (the
            # reference saturates through sigmoid(inf)=1).  Clamp the carry so
            # the next block's 0*carry segment reset never sees inf; any clamp
            # >= ~30 leaves sigmoid(c) exactly 1.0f.
            Ccl = sm.tile([HC, B], F16, tag="ccl")
            nc.vector.tensor_scalar_min(Ccl[:], r3(c[:])[:, :, tb - 1], CCLAMP)
            Cc_v = Ccl[:]
            Hc_v = r3(h[:])[:, :, tb - 1]
            nc.scalar.activation(ENc[:], Nc[:], AF.Exp, scale=-1.0)

    nc.compile()
    return nc


def _get_program():
    key = (S, TB)
    if key not in _cached:
        _cached[key] = build_program(S, TB)
    return _cached[key]


def host_inputs(x_codes, Wi_w, Wi_b, Wf_w, Wf_b, Wo_w, Wo_b, Wg_w, Wg_b,
                Wl_w, Wl_b, proj_w, proj_b, n_init):
    """Fold input normalization into per-gate ACT scale/bias; shard over H."""
    f = lambda v: np.asarray(v, np.float32)
    cols = []
    for (w, b) in ((Wi_w, Wi_b), (Wf_w, Wf_b), (Wo_w, Wo_b)):
        cols += [f(w) / 100.0, f(b) - 0.65 * f(w)]
    for (w, b) in ((Wg_w, Wg_b), (Wl_w, Wl_b)):
        cols += [f(w) / 200.0, (f(b) - 0.65 * f(w)) / 2.0]
    wv_full = np.stack(cols, axis=1).astype(np.float32)  # [H, 10]
    x = np.ascontiguousarray(f(x_codes)).astype(np.float16)
    pw = f(proj_w)
    n0 = f(n_init)
    maps = []
    for k in range(NCORES):
        hs = slice(k * HC, (k + 1) * HC)
        maps.append({
            "x": x,
            "wv": np.ascontiguousarray(wv_full[hs]),
            "projT": np.ascontiguousarray(pw[:, hs].T),
            "n0": np.ascontiguousarray(n0[hs].reshape(HC, 1)),
        })
    return maps


def assemble_output(results, proj_b, s=S, tb=TB):
    nb = s // tb
    nslab = (B * tb) // 128
    bper = 128 // tb  # batches per slab
    y = np.zeros((B, s, 2), np.float64)
    for k in range(NCORES):
        yc = np.asarray(results[k]["yout"], np.float64)
        ycr = yc.reshape(nb, bper, tb, nslab, 2)
        y += np.transpose(ycr, (3, 1, 0, 2, 4)).reshape(B, s, 2)
    y += np.asarray(proj_b, np.float64)[None, None, :]
    return y.astype(np.float32)


def kernel(**inputs):
    global _last_results
    nc = _get_program()
    maps = host_inputs(**inputs)
    res = run_bass_kernel_spmd(
        nc, maps, list(range(NCORES)),
        trace=bool(os.environ.get("KTRACE")),
        tmpdir=os.environ.get("KTRACE_DIR") or None,
    )
    _last_results = res
    return assemble_output(res.results, inputs["proj_b"])



# revision 5
# speedup vs baseline: 1.6164x; 1.6164x over previous
"""CfC head (mLSTM-style scan) Trainium2 kernel, v2.

Math (per timestep t, per (b,h)):
    pre_g = xt*Wg_w + Wg_b            (xt = (x_codes-65)/100)
    i_t = exp(pre_i - n), f_t = exp(pre_f - n), o_t = exp(pre_o - n)
    g_t = sigmoid(pre_g); lam = sigmoid(pre_l)
    c   = f_t*c + i_t*g_t
    h   = (h + DT*o_t*sigmoid(c)) / (1 + DT*lam)
    n  += 0.01*(i_t + f_t + o_t - 3)
    y_t = h @ proj_w.T + proj_b

Device mapping: H=1024 sharded over 8 cores (128 h-values per core, one SBUF
partition each); free dim packs (batch-major, time-minor) blocks of TB steps.

n-recurrence: instead of a per-step drift scan, n is held constant within a
block at the mid-block value.  Per block, Se = sum_t (Ei+Ef+Eo) (one DVE
reduce); with SP = Se*exp(-Nc) the self-consistent block update is
    dn = (0.01*SP - 0.03*TB) / (1 + 0.005*SP)
(the denominator linearizes the within-block feedback of n on the gates), and
the gates are scaled by EN = exp(-(Nc + dn/2)) (mid-block centering).
Validated vs reference in fp16-emulating numpy: rel err 1.4e-3 at TB=64
(budget 2e-2); ablations: no-midpoint 5.3e-3, no-selfconsistency 6.6e-3.

c and h are exact affine scans given EN:
    c_t = (Ef_t*EN) * c_{t-1} + (Ei_t*G_t*EN)
    h_t = L1_t * h_{t-1} + L1D_t*Eo_t*EN*(Tc_t+1),  L1 = 1/(1+DT*lam)
L1 uses the Neumann form 1 - q + q^2 = (q-0.5)^2 + 0.75 (q = DT*lam <= 0.01),
fp32 (its error is amplified ~200x as the h-scan decay rate).  L1D = DT/2*L1
uses the first-order form DT/2*(1-q), affine in Tl = tanh(pre_l/2), so it is
one fp16 tensor_scalar (the dropped q^2 term is <=1e-4 relative on the
additive b-term; validated).  Sigmoids use tanh so every activation
(exp/tanh/square/identity) stays in the single "exp_and_others" ACT table.

Projection: pj [128,2] is the stationary matmul operand; h streams as rhs in
512-column chunks (one PSUM bank each), out [2, B*TB] per block.  Partials
over the 8 cores are summed on the host.

Emission is software-pipelined: block k+1's gate ACTs are emitted on ScalarE
before block k's Tc, and block k+1's gate-dependent DVE head (G, L1D, EiG,
Esum, reduce, dn-chain) fills the DVE bubble while ScalarE computes Tc(k).
"""

import os
from contextlib import ExitStack

import numpy as np

import concourse.bacc as bacc
import concourse.mybir as mybir
import concourse.tile as tile
from concourse.bass_utils import run_bass_kernel_spmd

AF = mybir.ActivationFunctionType
OP = mybir.AluOpType
F32 = mybir.dt.float32
F16 = mybir.dt.float16

B, S, H = 64, 2048, 1024
NCORES = 8
HC = H // NCORES  # 128 h-values per core = partition dim
DT = 0.01

TB = int(os.environ.get("KERNEL_TB", "64"))  # timesteps per block
CCLAMP = 3.0e4  # c-carry clamp; sigmoid(c>=17) == 1.0f so this is exact

_cached = {}
_last_results = None


def build_program(s=S, tb=TB):
    nb = s // tb
    nfd = B * tb           # free dim of block tiles, (b-major, t-minor)
    mmc = 512              # matmul chunk: [2, 512] fp32 out = one PSUM bank
    nmm = nfd // mmc

    nc = bacc.Bacc(
        "TRN2", target_bir_lowering=False, debug=False, num_devices=NCORES
    )
    x_d = nc.dram_tensor("x", [B, s], F16, kind="ExternalInput").ap()
    wv_d = nc.dram_tensor("wv", [HC, 10], F32, kind="ExternalInput").ap()
    pj_d = nc.dram_tensor("projT", [HC, 2], F32, kind="ExternalInput").ap()
    n0_d = nc.dram_tensor("n0", [HC, 1], F32, kind="ExternalInput").ap()
    y_d = nc.dram_tensor("yout", [nb, 2, nfd], F32, kind="ExternalOutput").ap()

    def r3(ap):  # [128, nfd] -> [128, B, tb]
        return ap.rearrange("p (b t) -> p b t", t=tb)

    with tile.TileContext(nc) as tc, ExitStack() as ctx:
        wp = ctx.enter_context(tc.tile_pool(name="w", bufs=1))
        pha = ctx.enter_context(tc.tile_pool(name="pha", bufs=2))
        chn = ctx.enter_context(tc.tile_pool(name="chn", bufs=1))
        pp = ctx.enter_context(tc.tile_pool(name="pp", bufs=1, space="PSUM"))
        smp = ctx.enter_context(tc.tile_pool(name="smp", bufs=1))

        wv = wp.tile([HC, 10], F32)
        nc.sync.dma_start(wv[:], wv_d)
        pj = wp.tile([HC, 2], F32)
        nc.sync.dma_start(pj[:], pj_d)
        n0t = wp.tile([HC, 1], F32)
        nc.sync.dma_start(n0t[:], n0_d)

        # persistent state and per-block scratch (one buffer each)
        Nc = wp.tile([HC, B], F32)
        nc.vector.memset(Nc[:], 0.0)
        nc.vector.tensor_scalar(Nc[:], Nc[:], n0t[:, 0:1], None, OP.add)
        ENc0 = wp.tile([HC, B], F16)   # exp(-Nc) at block start
        nc.scalar.activation(ENc0[:], Nc[:], AF.Exp, scale=-1.0)
        ENc = wp.tile([HC, B], F16)    # exp(-(Nc + dn/2)) mid-block
        Ccl = wp.tile([HC, B], F16)    # clamped c carry
        nc.vector.memset(Ccl[:], 0.0)
        hz = wp.tile([HC, B], F32)     # zero h carry for block 0
        nc.vector.memset(hz[:], 0.0)
        bqm = wp.tile([HC, 1], F32)
        nc.vector.memset(bqm[:], DT / 2 - 0.5)
        b75 = wp.tile([HC, 1], F32)
        nc.vector.memset(b75[:], 0.75)
        Se = wp.tile([HC, B], F32)
        SPt = wp.tile([HC, B], F32)
        numt = wp.tile([HC, B], F32)
        dent = wp.tile([HC, B], F32)
        rdent = wp.tile([HC, B], F32)
        dnt = wp.tile([HC, B], F32)
        Nargt = wp.tile([HC, B], F32)
        t64 = wp.tile([HC, B], F16)
        t64b = wp.tile([HC, B], F32)

        # block-cycled tiles (single buffer; in-order engines keep them safe)
        ENcF = chn.tile([HC, nfd], F16, tag="ENcF")
        ct = chn.tile([HC, nfd], F16, tag="c")
        Tc = chn.tile([HC, nfd], F16, tag="Tc")
        L1D = chn.tile([HC, nfd], F16, tag="L1D")
        ht = chn.tile([HC, nfd], F32, tag="h")
        ps = pp.tile([2, nfd], F32)
        ysb = smp.tile([2, nfd], F32)

        def prep_sc(k):
            """DMA + gate ACTs for block k (ScalarE stream; tanh first so the
            DVE head can start before the exps finish)."""
            t0 = k * tb
            d = {}
            d["X"] = pha.tile([128, nfd], F16, tag="X", name="X")
            nc.sync.dma_start(
                r3(d["X"][:]), x_d[:, t0 : t0 + tb].partition_broadcast(128)
            )
            d["Tg"] = pha.tile([128, nfd], F16, tag="Tg", name="Tg")
            nc.scalar.activation(
                d["Tg"][:], d["X"][:], AF.Tanh, bias=wv[:, 7:8], scale=wv[:, 6:7]
            )
            d["Tl"] = pha.tile([128, nfd], F16, tag="Tl", name="Tl")
            nc.scalar.activation(
                d["Tl"][:], d["X"][:], AF.Tanh, bias=wv[:, 9:10], scale=wv[:, 8:9]
            )
            d["Ei"] = pha.tile([128, nfd], F16, tag="Ei", name="Ei")
            nc.scalar.activation(
                d["Ei"][:], d["X"][:], AF.Exp, bias=wv[:, 1:2], scale=wv[:, 0:1]
            )
            d["Ef"] = pha.tile([128, nfd], F16, tag="Ef", name="Ef")
            nc.scalar.activation(
                d["Ef"][:], d["X"][:], AF.Exp, bias=wv[:, 3:4], scale=wv[:, 2:3]
            )
            d["Eo"] = pha.tile([128, nfd], F16, tag="Eo", name="Eo")
            nc.scalar.activation(
                d["Eo"][:], d["X"][:], AF.Exp, bias=wv[:, 5:6], scale=wv[:, 4:5]
            )
            return d

        def prep_sq_sc(d):
            # SqL1 = (DT/2*Tl + (DT/2-0.5))^2 + 0.75 = 1 - q + q^2, fp32
            d["SqL1"] = pha.tile([128, nfd], F32, tag="SqL1", name="SqL1")
            nc.scalar.activation(
                d["SqL1"][:], d["Tl"][:], AF.Square, bias=bqm[:], scale=DT / 2
            )
            nc.scalar.activation(
                d["SqL1"][:], d["SqL1"][:], AF.Identity, bias=b75[:]
            )

        def prep_dve(d):
            """Gate-dependent DVE head: G, L1D, EiG, Esum, reduce, dn chain."""
            # G = 0.5*Tg+0.5 ; EiG = Ei*G (lands in Tg)
            nc.vector.tensor_scalar(d["Tg"][:], d["Tg"][:], 0.5, 0.5, OP.mult, OP.add)
            # L1D = DT/2*(1 - q) = -DT^2/4 * Tl + (DT/2 - DT^2/4)
            nc.vector.tensor_scalar(
                L1D[:], d["Tl"][:], -DT * DT / 4, DT / 2 - DT * DT / 4,
                OP.mult, OP.add,
            )
            nc.vector.tensor_mul(d["Tg"][:], d["Ei"][:], d["Tg"][:])
            # Esum = Ei+Ef+Eo (lands in Ei), then u = Eo*L1D (lands in Eo)
            nc.vector.tensor_add(d["Ei"][:], d["Ei"][:], d["Ef"][:])
            nc.vector.tensor_add(d["Ei"][:], d["Ei"][:], d["Eo"][:])
            nc.vector.tensor_mul(d["Eo"][:], d["Eo"][:], L1D[:])
            nc.vector.tensor_reduce(
                Se[:], r3(d["Ei"][:]), axis=mybir.AxisListType.X, op=OP.add
            )
            # dn = (0.01*SP - 0.03*tb)/(1 + 0.005*SP), SP = Se*exp(-Nc)
            nc.vector.tensor_mul(SPt[:], Se[:], ENc0[:])
            nc.vector.tensor_scalar(
                numt[:], SPt[:], 0.01, -0.03 * tb, OP.mult, OP.add
            )
            nc.vector.tensor_scalar(dent[:], SPt[:], 0.005, 1.0, OP.mult, OP.add)
            nc.vector.reciprocal(rdent[:], dent[:])
            nc.vector.tensor_mul(dnt[:], numt[:], rdent[:])
            nc.vector.scalar_tensor_tensor(
                Nargt[:], dnt[:], 0.5, Nc[:], OP.mult, OP.add
            )
            nc.vector.tensor_add(Nc[:], Nc[:], dnt[:])

        def prep_en_sc():
            nc.scalar.activation(ENc[:], Nargt[:], AF.Exp, scale=-1.0)
            nc.scalar.activation(ENc0[:], Nc[:], AF.Exp, scale=-1.0)

        def prep_encf():
            nc.vector.tensor_copy(
                r3(ENcF[:]), ENc[:].unsqueeze(2).broadcast_to([HC, B, tb])
            )

        # ---- prologue: full prep of block 0
        cur = prep_sc(0)
        prep_sq_sc(cur)
        prep_dve(cur)
        prep_en_sc()
        prep_encf()

        for k in range(nb):
            last = k == nb - 1
            if not last:
                nxt = prep_sc(k + 1)    # ScalarE: gates(k+1) before Tc(k)

            # c-scan coefficients: a_c = Ef*EN (in Ef), b_c = EiG*EN (in Tg)
            nc.vector.tensor_mul(cur["Ef"][:], cur["Ef"][:], ENcF[:])
            nc.vector.tensor_mul(cur["Tg"][:], cur["Tg"][:], ENcF[:])
            nc.vector.tensor_mul(t64[:], r3(cur["Ef"][:])[:, :, 0], Ccl[:])
            nc.vector.tensor_add(
                r3(cur["Tg"][:])[:, :, 0], r3(cur["Tg"][:])[:, :, 0], t64[:]
            )
            nc.vector.memset(r3(cur["Ef"][:])[:, :, 0], 0.0)
            nc.vector.tensor_tensor_scan(
                ct[:], cur["Ef"][:], cur["Tg"][:], 0.0, OP.mult, OP.add
            )

            nc.scalar.activation(Tc[:], ct[:], AF.Tanh, scale=0.5)

            if not last:                # DVE bubble-fill while ScalarE does Tc
                prep_sq_sc(nxt)
                prep_dve(nxt)
                prep_en_sc()

            # b_h = u*(Tc+1)*EN  (u = Eo*L1D, already in Eo)
            nc.vector.tensor_scalar(Tc[:], Tc[:], 1.0, None, OP.add)
            nc.vector.tensor_mul(cur["Eo"][:], cur["Eo"][:], Tc[:])
            nc.vector.tensor_mul(cur["Eo"][:], cur["Eo"][:], ENcF[:])
            if not last:
                prep_encf()             # EN broadcast for block k+1
            hprev = hz[:] if k == 0 else r3(ht[:])[:, :, tb - 1]
            nc.vector.tensor_mul(t64b[:], r3(cur["SqL1"][:])[:, :, 0], hprev)
            nc.vector.tensor_add(
                r3(cur["Eo"][:])[:, :, 0], r3(cur["Eo"][:])[:, :, 0], t64b[:]
            )
            nc.vector.memset(r3(cur["SqL1"][:])[:, :, 0], 0.0)
            nc.vector.tensor_tensor_scan(
                ht[:], cur["SqL1"][:], cur["Eo"][:], 0.0, OP.mult, OP.add
            )
            nc.vector.tensor_scalar_min(
                Ccl[:], r3(ct[:])[:, :, tb - 1], CCLAMP
            )

            # y partials: ps[2, nfd] = pj.T @ h, in one-PSUM-bank chunks
            for j in range(nmm):
                nc.tensor.matmul(
                    ps[:, j * mmc : (j + 1) * mmc],
                    pj[:],
                    ht[:, j * mmc : (j + 1) * mmc],
                    start=True,
                    stop=True,
                )
            nc.scalar.copy(ysb[:], ps[:])
            nc.sync.dma_start(y_d[k], ysb[:])

            if not last:
                cur = nxt

    nc.compile()
    return nc


def _get_program():
    key = (S, TB)
    if key not in _cached:
        _cached[key] = build_program(S, TB)
    return _cached[key]


def host_inputs(x_codes, Wi_w, Wi_b, Wf_w, Wf_b, Wo_w, Wo_b, Wg_w, Wg_b,
                Wl_w, Wl_b, proj_w, proj_b, n_init):
    """Fold input normalization into per-gate ACT scale/bias; shard over H."""
    f = lambda v: np.asarray(v, np.float32)
    cols = []
    for (w, b) in ((Wi_w, Wi_b), (Wf_w, Wf_b), (Wo_w, Wo_b)):
        cols += [f(w) / 100.0, f(b) - 0.65 * f(w)]
    for (w, b) in ((Wg_w, Wg_b), (Wl_w, Wl_b)):
        cols += [f(w) / 200.0, (f(b) - 0.65 * f(w)) / 2.0]
    wv_full = np.stack(cols, axis=1).astype(np.float32)  # [H, 10]
    x = np.ascontiguousarray(f(x_codes)).astype(np.float16)
    pw = f(proj_w)
    n0 = f(n_init)
    maps = []
    for k in range(NCORES):
        hs = slice(k * HC, (k + 1) * HC)
        maps.append({
            "x": x,
            "wv": np.ascontiguousarray(wv_full[hs]),
            "projT": np.ascontiguousarray(pw[:, hs].T),
            "n0": np.ascontiguousarray(n0[hs].reshape(HC, 1)),
        })
    return maps


def assemble_output(results, proj_b, s=S, tb=TB):
    nb = s // tb
    y = np.zeros((B, s, 2), np.float64)
    for k in range(NCORES):
        yc = np.asarray(results[k]["yout"], np.float64)  # [nb, 2, B*tb]
        ycr = yc.reshape(nb, 2, B, tb)
        y += np.transpose(ycr, (2, 0, 3, 1)).reshape(B, s, 2)
    y += np.asarray(proj_b, np.float64)[None, None, :]
    return y.astype(np.float32)


def kernel(**inputs):
    global _last_results
    nc = _get_program()
    maps = host_inputs(**inputs)
    res = run_bass_kernel_spmd(
        nc, maps, list(range(NCORES)),
        trace=bool(os.environ.get("KTRACE")),
        tmpdir=os.environ.get("KTRACE_DIR") or None,
    )
    _last_results = res
    return assemble_output(res.results, inputs["proj_b"])
